# revision 30
# baseline (speedup 1.0000x reference)
"""Trainium2 Bass kernel for multi-head self-attention.

Problem: B=4, T=2048, D=1024, H=16 heads (dh=64), causal, fp32 in/out.

Sharding (8 cores): core c -> (batch c % 4, head-group c // 4). Each core
computes one batch's 8 heads (tensor parallel over heads): QKV projection
for its head-group, attention, and a partial output projection (W_out
row-shard). The host sums the two head-group bf16 partials per batch and
adds b_out.

All matmul operands are bf16 (PSUM accumulation fp32). W_qkv/biases are
pre-arranged on the host into the on-chip layout so every weight load is
a large contiguous-row DMA; each dma_start costs ~3us fixed on its queue,
so traffic is batched into ~14 big transfers spread over the 3 DMA-capable
queues (sync/scalar/gpsimd).

Per core:
  Phase A: qkv^T = W^T x^T in 4 rounds of 512 t-columns; Q^T/K^T kept
           [d, t]; V kept [t, d] with an appended ones-column so the
           attention matmul also yields softmax denominators. Attention
           for query chunks qc=0..2 and the qc0 out-projection are
           interleaved under the QKV rounds so the (in-order) PE stream
           always has independent work. A short dummy-matmul burst warms
           the PE clock gate (HAM) during the startup DMAs.
  Attention (per head pair, query chunk of 512): scores^T[k, q] for the
           even/odd head land in the two halves of ONE wide PSUM tile
           (their 64-contraction matmuls occupy disjoint PE row groups and
           run concurrently); a single ACT exp covers both heads, and the
           2-wide psum pool then gives two key-tiles of lookahead so the
           scores stream is not in lockstep with exp. Causal diagonal
           blocks pre-load a -1e30 triangle into PSUM via a tiny
           identity-matmul (start=True) and accumulate scores on top -
           masking costs PE-only work, no extra DVE/Pool chain links.
           exp needs no max-subtraction (scores are O(1) by construction).
           y_aug^T = V_aug @ exp^T accumulated over key tiles; row 64 is
           the denominator; normalize on DVE (reciprocal+broadcast+mult).
  Phase B: qc=3 attention with the qc1/qc2/qc3 out-projections and their
           bf16 stores interleaved.
"""

import os
import sys

sys.path.insert(0, "/opt/trn_rl_repo")

import numpy as np

import concourse.bass as bass
import concourse.tile as tile
from concourse import bacc, mybir
from concourse.bass_utils import run_bass_kernel_spmd

B, T, D, H = 4, 2048, 1024, 16
DH = D // H          # 64
HL = H // 2          # 8 local heads per core
DL = HL * DH         # 512 local head dims
NT = T // 128        # 16 t-tiles of 128
NQC = T // 512       # 4 query chunks of 512
NKI = D // 128       # 8 contraction tiles for QKV

F32 = mybir.dt.float32
F32R = mybir.dt.float32r
BF16 = mybir.dt.bfloat16
EXPDT = BF16

_CACHED = {}


def _emit(nc, tc, causal):
    xT = nc.dram_tensors["xT"].ap()
    w_qkv = nc.dram_tensors["w_qkv"].ap()  # [128, 12288] pre-arranged
    b_qk_d = nc.dram_tensors["b_qk"].ap()  # [128, 8]
    b_v_d = nc.dram_tensors["b_v"].ap()    # [1, DL]
    w_out = nc.dram_tensors["w_out"].ap()
    out = nc.dram_tensors["out"].ap()

    xT_r = xT.rearrange("(ko ki) t -> ki ko t", ki=128)
    scale = float(1.0 / np.sqrt(DH))

    with (
        tc.tile_pool(name="const", bufs=1) as cpool,
        tc.tile_pool(name="qkvT", bufs=1) as qpool,
        tc.tile_pool(name="yT", bufs=1) as ypool,
        tc.tile_pool(name="expT", bufs=2) as epool,
        tc.tile_pool(name="rec", bufs=2) as rpool,
        tc.tile_pool(name="wo", bufs=1) as wopool,
        tc.tile_pool(name="ostg", bufs=2) as opool,
        tc.tile_pool(name="yp", bufs=2, space="PSUM") as yp,
    ):
        QT = qpool.tile([128, HL // 2, T], BF16, tag="QT")  # [d-pair, pair, t]
        KT = qpool.tile([128, HL // 2, T], BF16, tag="KT")
        V = qpool.tile([128, NT, HL, DH + 1], BF16, tag="V")
        yT = ypool.tile([128, HL // 2, T], BF16, tag="yT")

        # constants: additive causal mask for the diagonal 128x128 block
        # (0 where q_local >= k_local, -1e30 otherwise), ones, biases
        ones1 = cpool.tile([128, 1], F32, tag="ones1")
        nc.vector.memset(ones1[:], 1.0)
        idn = cpool.tile([128, 128], BF16, tag="idn")
        nc.vector.memset(idn[:], 1.0)
        nc.gpsimd.affine_select(
            out=idn[:],
            in_=idn[:],
            compare_op=mybir.AluOpType.is_equal,
            fill=0.0,
            base=0,
            pattern=[[1, 128]],
            channel_multiplier=-1,
        )
        amask = cpool.tile([128, 128], BF16, tag="amask")
        nc.vector.memset(amask[:], 0.0)
        nc.gpsimd.affine_select(
            out=amask[:],
            in_=amask[:],
            compare_op=mybir.AluOpType.is_ge,
            fill=-1e30,
            base=0,
            pattern=[[1, 128]],
            channel_multiplier=-1,
        )
        bqk_sb = cpool.tile([128, 8], F32, tag="bqk")
        nc.scalar.dma_start(bqk_sb[:], b_qk_d)
        bqk = [bqk_sb[:, c : c + 1] for c in range(8)]
        bv1 = cpool.tile([1, DL], F32, tag="bv1")
        nc.scalar.dma_start(bv1[:], b_v_d)
        bv = cpool.tile([128, DL], F32, tag="bv")
        nc.gpsimd.partition_broadcast(bv[:], bv1[:])

        sp_pool = [None]   # pair-scores psum pool
        ep_sel = [epool]   # eT pool (qc3 gets a double-buffered one)
        out_ps = [None]    # psum pool for out-proj tiles
        out_wide = [False]  # phase B: draw [128,1024] slots from the sp pool
        wo_sb = wopool.tile([128, HL // 2, D], BF16, tag="wo")

        out_r = out.rearrange("(a p) d -> p a d", p=128)

        def out_chunk(qci):
            """out rows for t-tiles 4*qci..4*qci+3 (needs yT cols qci*512..).
            One big bf16 store for the whole chunk."""
            stg = opool.tile([128, 4, D], BF16, tag="ostg", name=f"stg_{qci}")
            for ti in range(4):
                tt = 4 * qci + ti
                for n in range(2):
                    if out_wide[0]:
                        ps = out_ps[0].tile(
                            [128, 1024], F32, tag="sp", name=f"psC_{tt}_{n}"
                        )[:, :512]
                    else:
                        ps = out_ps[0].tile(
                            [128, 512], F32, tag="psA", name=f"psC_{tt}_{n}"
                        )
                    for j in range(HL // 2):
                        nc.tensor.matmul(
                            ps[:],
                            yT[:, j, tt * 128 : (tt + 1) * 128],
                            wo_sb[:, j, n * 512 : (n + 1) * 512],
                            start=(j == 0),
                            stop=(j == HL // 2 - 1),
                        )
                    nc.vector.tensor_copy(
                        stg[:, ti, n * 512 : (n + 1) * 512], ps[:]
                    )
            eng = [nc.sync, nc.scalar, nc.gpsimd][qci % 3]
            eng.dma_start(out_r[:, 4 * qci : 4 * qci + 4, :], stg[:])

        def scores_pair(p, qc, pairs=True):
            """Scores+exp for head pair (2p, 2p+1). Per key-tile the two
            64-contraction matmuls land in disjoint PE row-groups (HW runs
            them concurrently) and write the two halves of ONE wide psum
            tile, so a single exp covers both heads and the psum pool gives
            two key-tiles of lookahead."""
            QT0 = QT[0:64, p]
            QT1 = QT[64:128, p]
            KT0 = KT[0:64, p]
            KT1 = KT[64:128, p]
            nkt = 4 * qc + 4 if causal else NT
            ndiag = 4 if causal else 0
            qlo = qc * 512
            eP = ep_sel[0].tile(
                [128, NT, 2, 512], EXPDT, tag="eP", name=f"eP_{p}_{qc}"
            )
            for kt in range(nkt - ndiag):  # non-diagonal key tiles
                ps = sp_pool[0].tile(
                    [128, 1024], F32, tag="sp", name=f"sp_{p}_{qc}_{kt}"
                )
                nc.tensor.matmul(
                    ps[:, 0:512],
                    KT0[:, kt * 128 : (kt + 1) * 128],
                    QT0[:, qlo : qlo + 512],
                    start=True,
                    stop=True,
                )
                nc.tensor.matmul(
                    ps[:, 512:1024],
                    KT1[:, kt * 128 : (kt + 1) * 128],
                    QT1[:, qlo : qlo + 512],
                    start=True,
                    stop=True,
                )
                nc.scalar.activation(
                    eP[:, kt],
                    ps.rearrange("p (a b) -> p a b", a=2),
                    mybir.ActivationFunctionType.Exp,
                    scale=scale,
                )
            for r in range(ndiag):  # diagonal tiles: exp then zero triangle
                kt = (4 * qc + r) if causal else (nkt - ndiag + r)
                valid = 512 - r * 128
                ps = sp_pool[0].tile(
                    [128, 1024], F32, tag="sp", name=f"spd_{p}_{qc}_{r}"
                )
                # -1e30 triangle into the boundary 128 cols of each half
                # (start=True sets has_written there); the scores matmuls
                # then accumulate on the triangle and overwrite elsewhere.
                nc.tensor.matmul(
                    ps[:, 0:128], idn[:], amask[:],
                    start=True, stop=False, skip_group_check=True,
                )
                nc.tensor.matmul(
                    ps[:, 512:640], idn[:], amask[:],
                    start=True, stop=False, skip_group_check=True,
                )
                nc.tensor.matmul(
                    ps[:, 0:valid],
                    KT0[:, kt * 128 : (kt + 1) * 128],
                    QT0[:, qlo + r * 128 : qlo + 512],
                    start=False,
                    stop=True,
                    skip_group_check=True,
                )
                nc.tensor.matmul(
                    ps[:, 512 : 512 + valid],
                    KT1[:, kt * 128 : (kt + 1) * 128],
                    QT1[:, qlo + r * 128 : qlo + 512],
                    start=False,
                    stop=True,
                    skip_group_check=True,
                )
                nc.scalar.activation(
                    eP[:, kt, :, r * 128 :],
                    ps.rearrange("p (a b) -> p a b", a=2)[:, :, :valid],
                    mybir.ActivationFunctionType.Exp,
                    scale=scale,
                )
            return eP

        def av_part(h, qc, eP):
            par = h % 2
            pj = h // 2
            nkt = 4 * qc + 4 if causal else NT
            qlo = qc * 512
            # attention @ V_aug; diagonal kts only touch their valid
            # q-columns (invalid eP regions are never read)
            yps = yp.tile([65, 512], F32, tag="yp", name=f"yp_{h}_{qc}")
            for kt in range(nkt):
                r = kt - 4 * qc if (causal and kt >= 4 * qc) else 0
                nc.tensor.matmul(
                    yps[:, r * 128 :],
                    V[:, kt, h, :],
                    eP[:, kt, par, r * 128 :],
                    start=(kt == 0),
                    stop=(kt == nkt - 1),
                )
            rec = rpool.tile([1, 512], F32, tag="rec")
            nc.vector.reciprocal(rec[:], yps[64:65, :])
            rbc = rpool.tile([64, 512], F32, tag="rbc")
            nc.gpsimd.partition_broadcast(rbc[:], rec[:])
            nc.vector.tensor_tensor(
                yT[par * 64 : par * 64 + 64, pj, qlo : qlo + 512],
                yps[:64, :],
                rbc[:],
                mybir.AluOpType.mult,
            )

        def attn_chunk(qc, pairs=True):
            prev = scores_pair(0, qc)
            for p in range(HL // 2):
                nxt = scores_pair(p + 1, qc) if p + 1 < HL // 2 else None
                av_part(2 * p, qc, prev)
                av_part(2 * p + 1, qc, prev)
                prev = nxt

        # ---- Phase A (+ attention qc=0..2 interleaved) ----
        with (
            tc.tile_pool(name="xw", bufs=1) as wpool,
            tc.tile_pool(name="xstream", bufs=2) as xwpool,
            tc.tile_pool(name="psA", bufs=2, space="PSUM") as psA,
            tc.tile_pool(name="spdp", bufs=2, space="PSUM") as spd,
        ):
            sp_pool[0] = spd
            wq_sb = wpool.tile([128, 12288], BF16, tag="wq")
            # startup: x round 0 first on gpsimd so QKV starts ~max(w0,x0);
            # V-weight halves ride behind the Q/K chunks on sync/scalar.
            xc0 = xwpool.tile([128, NKI, 512], BF16, tag="xc", name="xc_r0")
            nc.gpsimd.dma_start(xc0[:], xT_r[:, :, 0:512])
            nc.sync.dma_start(wq_sb[:, 0:4096], w_qkv[:, 0:4096])
            nc.scalar.dma_start(wq_sb[:, 4096:8192], w_qkv[:, 4096:8192])
            nc.sync.dma_start(wq_sb[:, 8192:10240], w_qkv[:, 8192:10240])
            nc.scalar.dma_start(wq_sb[:, 10240:12288], w_qkv[:, 10240:12288])
            nc.gpsimd.dma_start(
                wo_sb[:], w_out.rearrange("(j p) d -> p j d", p=128)
            )
            xc_tiles = {0: xc0}
            xc_engs = [None, nc.sync, nc.scalar, nc.gpsimd]
            # HAM warm-up: keep PE busy during the startup DMAs so the
            # clock gate reaches 8/8 before the real stream begins.
            for wrm in range(10):
                wps = psA.tile([128, 512], F32, tag="psA", name=f"warm_{wrm}")
                for _ in range(4):
                    nc.tensor.matmul(
                        wps[:, :128], idn[:], idn[:],
                        start=True, stop=True, skip_group_check=True,
                    )

            def wch(c, kt):
                return wq_sb[:, c * 1024 + kt * 128 : c * 1024 + (kt + 1) * 128]

            def wv_chunk(kt):
                return wq_sb[:, 2 * DL * 8 + kt * DL : 2 * DL * 8 + (kt + 1) * DL]

            def tc_round(tcx):
                if tcx in xc_tiles:
                    xc = xc_tiles[tcx]
                else:
                    xc = xwpool.tile(
                        [128, NKI, 512], BF16, tag="xc", name=f"xc_r{tcx}"
                    )
                    xc_engs[tcx].dma_start(
                        xc[:], xT_r[:, :, tcx * 512 : (tcx + 1) * 512]
                    )
                for c in range(8):  # Q/K channel tiles
                    dstT = QT if c < 4 else KT
                    ps = psA.tile([128, 512], F32, tag="psA")
                    for kt in range(NKI):
                        nc.tensor.matmul(
                            ps[:],
                            wch(c, kt),
                            xc[:, kt],
                            start=(kt == 0),
                            stop=(kt == NKI - 1),
                        )
                    nc.vector.tensor_scalar_add(
                        dstT[:, c % 4, tcx * 512 : (tcx + 1) * 512],
                        ps[:],
                        bqk[c],
                    )
                for tt in range(4 * tcx, 4 * tcx + 4):  # V t-tiles
                    ps2 = psA.tile([128, DL], F32, tag="psA")
                    for kt in range(NKI):
                        nc.tensor.matmul(
                            ps2[:],
                            xc[:, kt, (tt % 4) * 128 : (tt % 4 + 1) * 128],
                            wv_chunk(kt),
                            start=(kt == 0),
                            stop=(kt == NKI - 1),
                        )
                    nc.vector.tensor_tensor(
                        V[:, tt, :, :DH],
                        ps2.rearrange("p (h d) -> p h d", h=HL),
                        bv.rearrange("p (h d) -> p h d", h=HL),
                        mybir.AluOpType.add,
                    )
                    nc.vector.tensor_copy(
                        V[:, tt, :, DH], ones1.to_broadcast((128, HL))
                    )

            out_ps[0] = psA
            tc_round(0)
            if causal:
                attn_chunk(0)  # qc0 only needs the diag psum pool
            tc_round(1)
            if causal:
                out_chunk(0)
                attn_chunk(1)
            tc_round(2)
            if causal:
                attn_chunk(2)
            tc_round(3)

        # ---- Phases B (qc=3) + C interleaved ----
        with (
            tc.tile_pool(name="spp", bufs=3, space="PSUM") as sp,
        ):
            sp_pool[0] = sp
            out_ps[0] = sp
            out_wide[0] = True

            if not causal:
                attn_chunk(0)
                attn_chunk(1)
                attn_chunk(2)
                out_chunk(0)
                out_chunk(1)
                out_chunk(2)
            prev = scores_pair(0, 3)
            for p in range(HL // 2):
                nxt = scores_pair(p + 1, 3) if p + 1 < HL // 2 else None
                av_part(2 * p, 3, prev)
                av_part(2 * p + 1, 3, prev)
                if causal and p == 0:
                    out_chunk(1)
                if causal and p == 2:
                    out_chunk(2)
                prev = nxt
            out_chunk(3)


def _build(causal: bool, repeat: int = 1):
    nc = bacc.Bacc("TRN2", target_bir_lowering=False, debug=False)
    nc.dram_tensors = {}
    nc.dram_tensors["xT"] = nc.dram_tensor("xT", [D, T], BF16, kind="ExternalInput")
    nc.dram_tensors["w_qkv"] = nc.dram_tensor(
        "w_qkv", [128, 12288], BF16, kind="ExternalInput"
    )
    nc.dram_tensors["b_qk"] = nc.dram_tensor(
        "b_qk", [128, 8], F32, kind="ExternalInput"
    )
    nc.dram_tensors["b_v"] = nc.dram_tensor("b_v", [1, DL], F32, kind="ExternalInput")
    nc.dram_tensors["w_out"] = nc.dram_tensor(
        "w_out", [DL, D], BF16, kind="ExternalInput"
    )
    nc.dram_tensors["out"] = nc.dram_tensor("out", [T, D], BF16, kind="ExternalOutput")
    with tile.TileContext(nc) as tc:
        for _rep in range(repeat):
            _emit(nc, tc, causal)
    nc.compile()
    return nc


def _get_program(causal: bool):
    key = ("prog", causal)
    if key not in _CACHED:
        _CACHED[key] = _build(causal)
    return _CACHED[key]


def _run_fast(nc, causal, in_maps):
    """Execute via a cached jitted shard_map executable (avoids rebuilding
    the PJRT program on every call). Falls back to run_bass_kernel_spmd."""
    try:
        import jax
        from jax.sharding import Mesh, NamedSharding, PartitionSpec
        from jax.experimental.shard_map import shard_map
        from concourse import bass2jax
        from concourse.bass2jax import _bass_exec_p, install_neuronx_cc_hook

        key = ("exec", causal)
        if key not in _CACHED:
            install_neuronx_cc_hook()
            partition_name = (
                nc.partition_id_tensor.name if nc.partition_id_tensor else None
            )
            in_names, out_names, out_avals, zero_outs = [], [], [], []
            for alloc in nc.m.functions[0].allocations:
                if not isinstance(alloc, mybir.MemoryLocationSet):
                    continue
                name = alloc.memorylocations[0].name
                if alloc.kind == "ExternalInput":
                    if name != partition_name:
                        in_names.append(name)
                elif alloc.kind == "ExternalOutput":
                    out_names.append(name)
                    shape = tuple(alloc.tensor_shape)
                    dtype = mybir.dt.np(alloc.dtype)
                    out_avals.append(jax.core.ShapedArray(shape, dtype))
                    zero_outs.append(np.zeros(shape, dtype))
            n_params = len(in_names)
            in_names_full = in_names + out_names + (
                [partition_name] if partition_name else []
            )

            def _body(*args):
                operands = list(args)
                if partition_name is not None:
                    operands.append(bass2jax.partition_id_tensor())
                return tuple(
                    _bass_exec_p.bind(
                        *operands,
                        out_avals=tuple(out_avals),
                        in_names=tuple(in_names_full),
                        out_names=tuple(out_names),
                        lowering_input_output_aliases=(),
                        sim_require_finite=True,
                        sim_require_nnan=True,
                        nc=nc,
                    )
                )

            devices = jax.devices()[:8]
            mesh = Mesh(np.asarray(devices), ("core",))
            ex = jax.jit(
                shard_map(
                    _body,
                    mesh=mesh,
                    in_specs=(PartitionSpec("core"),) * (n_params + len(out_names)),
                    out_specs=(PartitionSpec("core"),) * len(out_names),
                    check_rep=False,
                ),
                keep_unused=True,
            )
            _CACHED[key] = (ex, in_names, zero_outs, mesh)
        ex, in_names, zero_outs, mesh = _CACHED[key]
        sh = NamedSharding(mesh, PartitionSpec("core"))
        concat_in = [
            np.concatenate([np.asarray(m[nm]) for m in in_maps], axis=0)
            for nm in in_names
        ]
        concat_zeros = [
            np.zeros((8 * z.shape[0], *z.shape[1:]), z.dtype) for z in zero_outs
        ]
        dev = [jax.device_put(a, sh) for a in concat_in + concat_zeros]
        out_arrs = ex(*dev)
        full = np.asarray(out_arrs[0]).reshape(8, T, D)
        return [full[c] for c in range(8)]
    except Exception:
        res = run_bass_kernel_spmd(nc, in_maps, list(range(8)))
        return [r["out"] for r in res.results]


def kernel(x, attn_mask, W_qkv, b_qkv, W_out, b_out, causal):
    from concourse import mybir as _mybir

    bf16 = _mybir.dt.np(_mybir.dt.bfloat16)
    x = np.asarray(x, dtype=np.float32)
    W_qkv = np.asarray(W_qkv, dtype=np.float32)
    b_qkv_np = np.asarray(b_qkv, dtype=np.float32)
    W_out = np.asarray(W_out, dtype=np.float32)
    b_out = np.asarray(b_out, dtype=np.float32)
    causal = bool(int(causal))

    nc = _get_program(causal)

    shards = []
    for g in range(2):
        w_shard = np.ascontiguousarray(
            np.concatenate(
                [
                    W_qkv[:, g * DL : (g + 1) * DL],
                    W_qkv[:, D + g * DL : D + (g + 1) * DL],
                    W_qkv[:, 2 * D + g * DL : 2 * D + (g + 1) * DL],
                ],
                axis=1,
            )
        )
        b_shard = np.ascontiguousarray(
            np.concatenate(
                [
                    b_qkv_np[g * DL : (g + 1) * DL],
                    b_qkv_np[D + g * DL : D + (g + 1) * DL],
                    b_qkv_np[2 * D + g * DL : 2 * D + (g + 1) * DL],
                ]
            )
        )
        # pre-arrange W_qkv into the on-chip layout: [ki, c, ko, 128] for the
        # 8 Q/K channel tiles, then [ki, ko, 512] for V (2KB-contiguous rows)
        ws3 = w_shard.reshape(8, 128, 3 * DL)
        qk = (
            ws3[:, :, : 2 * DL]
            .reshape(8, 128, 8, 128)
            .transpose(1, 2, 0, 3)
            .reshape(128, 2 * DL * 8)
        )
        vpart = ws3[:, :, 2 * DL :].transpose(1, 0, 2).reshape(128, DL * 8)
        warr = np.ascontiguousarray(np.concatenate([qk, vpart], axis=1)).astype(bf16)
        b_qk_arr = np.ascontiguousarray(b_shard[: 2 * DL].reshape(8, 128).T).astype(
            np.float32
        )
        b_v_arr = np.ascontiguousarray(b_shard[2 * DL :].reshape(1, DL)).astype(
            np.float32
        )
        wo_shard = np.ascontiguousarray(W_out[g * DL : (g + 1) * DL, :]).astype(bf16)
        shards.append((warr, b_qk_arr, b_v_arr, wo_shard))

    in_maps = []
    for c in range(8):
        b = c % B
        g = c // B
        warr, b_qk_arr, b_v_arr, wo_shard = shards[g]
        in_maps.append(
            {
                "xT": np.ascontiguousarray(x[b].T).astype(bf16),
                "w_qkv": warr,
                "b_qk": b_qk_arr,
                "b_v": b_v_arr,
                "w_out": wo_shard,
            }
        )

    outs = _run_fast(nc, causal, in_maps)
    y = np.empty((B, T, D), dtype=np.float32)
    for b in range(B):
        y[b] = (
            np.asarray(outs[b], dtype=np.float32)
            + np.asarray(outs[B + b], dtype=np.float32)
            + b_out
        )
    return y



# revision 32
# speedup vs baseline: 1.2051x; 1.2051x over previous
"""Trainium2 Bass kernel for multi-head self-attention.

Problem: B=4, T=2048, D=1024, H=16 heads (dh=64), causal, fp32 in/out.

Sharding (8 cores): core c -> (batch c % 4, head-group c // 4). Each core
computes one batch's 8 heads (tensor parallel over heads): QKV projection
for its head-group, attention, and a partial output projection (W_out
row-shard). The host sums the two head-group bf16 partials per batch and
adds b_out.

All matmul operands are bf16 (PSUM accumulation fp32). W_qkv/biases are
pre-arranged on the host into the on-chip layout so every weight load is
a large contiguous-row DMA; each dma_start costs ~3us fixed on its queue,
so traffic is batched into ~14 big transfers spread over the 3 DMA-capable
queues (sync/scalar/gpsimd).

Per core:
  Phase A: qkv^T = W^T x^T in 4 rounds of 512 t-columns; Q^T/K^T kept
           [d, t]; V kept [t, d] with an appended ones-column so the
           attention matmul also yields softmax denominators. Attention
           for query chunks qc=0..2 and the qc0 out-projection are
           interleaved under the QKV rounds so the (in-order) PE stream
           always has independent work. A short dummy-matmul burst warms
           the PE clock gate (HAM) during the startup DMAs.
  Attention (per head pair, query chunk of 512): scores^T[k, q] for the
           even/odd head land in the two halves of ONE wide PSUM tile
           (their 64-contraction matmuls occupy disjoint PE row groups and
           run concurrently); a single ACT exp covers both heads, and the
           2-wide psum pool then gives two key-tiles of lookahead so the
           scores stream is not in lockstep with exp. Causal diagonal
           blocks pre-load a -1e30 triangle into PSUM via a tiny
           identity-matmul (start=True) and accumulate scores on top -
           masking costs PE-only work, no extra DVE/Pool chain links.
           exp needs no max-subtraction (scores are O(1) by construction).
           y_aug^T = V_aug @ exp^T accumulated over key tiles; row 64 is
           the denominator; normalize on DVE (reciprocal+broadcast+mult).
  Phase B: qc=3 attention with the qc1/qc2/qc3 out-projections and their
           bf16 stores interleaved.
"""

import os
import sys

sys.path.insert(0, "/opt/trn_rl_repo")

import numpy as np

import concourse.bass as bass
import concourse.tile as tile
from concourse import bacc, mybir
from concourse.bass_utils import run_bass_kernel_spmd

B, T, D, H = 4, 2048, 1024, 16
DH = D // H          # 64
HL = H // 2          # 8 local heads per core
DL = HL * DH         # 512 local head dims
NT = T // 128        # 16 t-tiles of 128
NQC = T // 512       # 4 query chunks of 512
NKI = D // 128       # 8 contraction tiles for QKV

F32 = mybir.dt.float32
F32R = mybir.dt.float32r
BF16 = mybir.dt.bfloat16
EXPDT = BF16

_CACHED = {}


def _emit(nc, tc, causal):
    xT = nc.dram_tensors["xT"].ap()
    w_qkv = nc.dram_tensors["w_qkv"].ap()  # [128, 12288] pre-arranged
    b_qk_d = nc.dram_tensors["b_qk"].ap()  # [128, 8]
    b_v_d = nc.dram_tensors["b_v"].ap()    # [1, DL]
    w_out = nc.dram_tensors["w_out"].ap()
    out = nc.dram_tensors["out"].ap()

    xT_r = xT.rearrange("(ko ki) t -> ki ko t", ki=128)
    scale = float(1.0 / np.sqrt(DH))

    with (
        tc.tile_pool(name="const", bufs=1) as cpool,
        tc.tile_pool(name="qkvT", bufs=1) as qpool,
        tc.tile_pool(name="yT", bufs=1) as ypool,
        tc.tile_pool(name="expT", bufs=2) as epool,
        tc.tile_pool(name="rec", bufs=2) as rpool,
        tc.tile_pool(name="wo", bufs=1) as wopool,
        tc.tile_pool(name="ostg", bufs=2) as opool,
        tc.tile_pool(name="yp", bufs=2, space="PSUM") as yp,
    ):
        QT = qpool.tile([128, HL // 2, T], BF16, tag="QT")  # [d-pair, pair, t]
        KT = qpool.tile([128, HL // 2, T], BF16, tag="KT")
        V = qpool.tile([128, NT, HL, DH + 1], BF16, tag="V")
        yT = ypool.tile([128, HL // 2, T], BF16, tag="yT")

        # constants: additive causal mask for the diagonal 128x128 block
        # (0 where q_local >= k_local, -1e30 otherwise), ones, biases
        ones1 = cpool.tile([128, 1], F32, tag="ones1")
        nc.vector.memset(ones1[:], 1.0)
        idn = cpool.tile([128, 128], BF16, tag="idn")
        nc.vector.memset(idn[:], 1.0)
        nc.gpsimd.affine_select(
            out=idn[:],
            in_=idn[:],
            compare_op=mybir.AluOpType.is_equal,
            fill=0.0,
            base=0,
            pattern=[[1, 128]],
            channel_multiplier=-1,
        )
        amask = cpool.tile([128, 128], BF16, tag="amask")
        nc.vector.memset(amask[:], 0.0)
        nc.gpsimd.affine_select(
            out=amask[:],
            in_=amask[:],
            compare_op=mybir.AluOpType.is_ge,
            fill=-1e30,
            base=0,
            pattern=[[1, 128]],
            channel_multiplier=-1,
        )
        bqk_sb = cpool.tile([128, 8], F32, tag="bqk")
        nc.scalar.dma_start(bqk_sb[:], b_qk_d)
        bqk = [bqk_sb[:, c : c + 1] for c in range(8)]
        bv1 = cpool.tile([1, DL], F32, tag="bv1")
        nc.scalar.dma_start(bv1[:], b_v_d)
        bv = cpool.tile([128, DL], F32, tag="bv")
        nc.gpsimd.partition_broadcast(bv[:], bv1[:])

        sp_pool = [None]   # pair-scores psum pool
        ep_sel = [epool]   # eT pool (qc3 gets a double-buffered one)
        out_ps = [None]    # psum pool for out-proj tiles
        out_tag = ["psA"]  # tag inside that pool (shares space with owner)
        wo_sb = wopool.tile([128, HL // 2, D], BF16, tag="wo")

        out_r = out.rearrange("(a p) d -> p a d", p=128)

        def out_chunk(qci):
            """out rows for t-tiles 4*qci..4*qci+3 (needs yT cols qci*512..).
            One big bf16 store for the whole chunk."""
            stg = opool.tile([128, 4, D], BF16, tag="ostg", name=f"stg_{qci}")
            for ti in range(4):
                tt = 4 * qci + ti
                for n in range(2):
                    ps = out_ps[0].tile(
                        [128, 512], F32, tag=out_tag[0], name=f"psC_{tt}_{n}"
                    )
                    for j in range(HL // 2):
                        nc.tensor.matmul(
                            ps[:],
                            yT[:, j, tt * 128 : (tt + 1) * 128],
                            wo_sb[:, j, n * 512 : (n + 1) * 512],
                            start=(j == 0),
                            stop=(j == HL // 2 - 1),
                        )
                    nc.vector.tensor_copy(
                        stg[:, ti, n * 512 : (n + 1) * 512], ps[:]
                    )
            eng = [nc.sync, nc.scalar, nc.gpsimd][qci % 3]
            eng.dma_start(out_r[:, 4 * qci : 4 * qci + 4, :], stg[:])

        def scores_pair(p, qc, pairs=True):
            """Scores+exp for head pair (2p, 2p+1). Per key-tile the two
            64-contraction matmuls land in disjoint PE row-groups (HW runs
            them concurrently) and write the two halves of ONE wide psum
            tile, so a single exp covers both heads and the psum pool gives
            two key-tiles of lookahead."""
            QT0 = QT[0:64, p]
            QT1 = QT[64:128, p]
            KT0 = KT[0:64, p]
            KT1 = KT[64:128, p]
            nkt = 4 * qc + 4 if causal else NT
            ndiag = 4 if causal else 0
            qlo = qc * 512
            eP = ep_sel[0].tile(
                [128, NT, 2, 512], EXPDT, tag="eP", name=f"eP_{p}_{qc}"
            )
            for kt in range(nkt - ndiag):  # non-diagonal key tiles
                ps = sp_pool[0].tile(
                    [128, 1024], F32, tag="sp", name=f"sp_{p}_{qc}_{kt}"
                )
                nc.tensor.matmul(
                    ps[:, 0:512],
                    KT0[:, kt * 128 : (kt + 1) * 128],
                    QT0[:, qlo : qlo + 512],
                    start=True,
                    stop=True,
                )
                nc.tensor.matmul(
                    ps[:, 512:1024],
                    KT1[:, kt * 128 : (kt + 1) * 128],
                    QT1[:, qlo : qlo + 512],
                    start=True,
                    stop=True,
                )
                nc.scalar.activation(
                    eP[:, kt],
                    ps.rearrange("p (a b) -> p a b", a=2),
                    mybir.ActivationFunctionType.Exp,
                    scale=scale,
                )
            for r in range(ndiag):  # diagonal tiles: exp then zero triangle
                kt = (4 * qc + r) if causal else (nkt - ndiag + r)
                valid = 512 - r * 128
                ps = sp_pool[0].tile(
                    [128, 1024], F32, tag="sp", name=f"spd_{p}_{qc}_{r}"
                )
                # -1e30 triangle into the boundary 128 cols of each half
                # (start=True sets has_written there); the scores matmuls
                # then accumulate on the triangle and overwrite elsewhere.
                nc.tensor.matmul(
                    ps[:, 0:128], idn[:], amask[:],
                    start=True, stop=False, skip_group_check=True,
                )
                nc.tensor.matmul(
                    ps[:, 512:640], idn[:], amask[:],
                    start=True, stop=False, skip_group_check=True,
                )
                nc.tensor.matmul(
                    ps[:, 0:valid],
                    KT0[:, kt * 128 : (kt + 1) * 128],
                    QT0[:, qlo + r * 128 : qlo + 512],
                    start=False,
                    stop=True,
                    skip_group_check=True,
                )
                nc.tensor.matmul(
                    ps[:, 512 : 512 + valid],
                    KT1[:, kt * 128 : (kt + 1) * 128],
                    QT1[:, qlo + r * 128 : qlo + 512],
                    start=False,
                    stop=True,
                    skip_group_check=True,
                )
                nc.scalar.activation(
                    eP[:, kt, :, r * 128 :],
                    ps.rearrange("p (a b) -> p a b", a=2)[:, :, :valid],
                    mybir.ActivationFunctionType.Exp,
                    scale=scale,
                )
            return eP

        av_pools = [None]  # (even-head pool, odd-head pool)

        def av_part(h, qc, eP):
            par = h % 2
            pj = h // 2
            nkt = 4 * qc + 4 if causal else NT
            qlo = qc * 512
            pool = av_pools[0][par] if av_pools[0] else yp
            # attention @ V_aug; diagonal kts only touch their valid
            # q-columns (invalid eP regions are never read)
            yps = pool.tile([65, 512], F32, tag="yp", name=f"yp_{h}_{qc}")
            for kt in range(nkt):
                r = kt - 4 * qc if (causal and kt >= 4 * qc) else 0
                nc.tensor.matmul(
                    yps[:, r * 128 :],
                    V[:, kt, h, :],
                    eP[:, kt, par, r * 128 :],
                    start=(kt == 0),
                    stop=(kt == nkt - 1),
                )
            rec = rpool.tile([1, 512], F32, tag="rec")
            nc.vector.reciprocal(rec[:], yps[64:65, :])
            rbc = rpool.tile([64, 512], F32, tag="rbc")
            nc.gpsimd.partition_broadcast(rbc[:], rec[:])
            nc.vector.tensor_tensor(
                yT[par * 64 : par * 64 + 64, pj, qlo : qlo + 512],
                yps[:64, :],
                rbc[:],
                mybir.AluOpType.mult,
            )

        def attn_chunk(qc, pairs=True):
            prev = scores_pair(0, qc)
            for p in range(HL // 2):
                nxt = scores_pair(p + 1, qc) if p + 1 < HL // 2 else None
                av_part(2 * p, qc, prev)
                av_part(2 * p + 1, qc, prev)
                prev = nxt

        # ---- Phase A (+ attention qc=0..2 interleaved) ----
        with (
            tc.tile_pool(name="xw", bufs=1) as wpool,
            tc.tile_pool(name="xstream", bufs=2) as xwpool,
            tc.tile_pool(name="psA", bufs=2, space="PSUM") as psA,
            tc.tile_pool(name="spdp", bufs=2, space="PSUM") as spd,
        ):
            sp_pool[0] = spd
            wq_sb = wpool.tile([128, 12288], BF16, tag="wq")
            # startup: x round 0 first on gpsimd so QKV starts ~max(w0,x0);
            # V-weight halves ride behind the Q/K chunks on sync/scalar.
            xc0 = xwpool.tile([128, NKI, 512], BF16, tag="xc", name="xc_r0")
            nc.gpsimd.dma_start(xc0[:, : NKI // 2], xT_r[:, : NKI // 2, 0:512])
            nc.sync.dma_start(
                xc0[:, NKI // 2 :], xT_r[:, NKI // 2 :, 0:512]
            )
            nc.sync.dma_start(wq_sb[:, 0:4096], w_qkv[:, 0:4096])
            nc.scalar.dma_start(wq_sb[:, 4096:8192], w_qkv[:, 4096:8192])
            nc.sync.dma_start(wq_sb[:, 8192:10240], w_qkv[:, 8192:10240])
            nc.scalar.dma_start(wq_sb[:, 10240:12288], w_qkv[:, 10240:12288])
            nc.gpsimd.dma_start(
                wo_sb[:], w_out.rearrange("(j p) d -> p j d", p=128)
            )
            xc_tiles = {0: xc0}
            xc_engs = [None, nc.sync, nc.scalar, nc.gpsimd]
            # HAM warm-up: keep PE busy during the startup DMAs so the
            # clock gate reaches 8/8 before the real stream begins.
            for wrm in range(10):
                wps = psA.tile([128, 512], F32, tag="psA", name=f"warm_{wrm}")
                for _ in range(4):
                    nc.tensor.matmul(
                        wps[:, :128], idn[:], idn[:],
                        start=True, stop=True, skip_group_check=True,
                    )

            def wch(c, kt):
                return wq_sb[:, c * 1024 + kt * 128 : c * 1024 + (kt + 1) * 128]

            def wv_chunk(kt):
                return wq_sb[:, 2 * DL * 8 + kt * DL : 2 * DL * 8 + (kt + 1) * DL]

            def tc_round(tcx):
                if tcx in xc_tiles:
                    xc = xc_tiles[tcx]
                else:
                    xc = xwpool.tile(
                        [128, NKI, 512], BF16, tag="xc", name=f"xc_r{tcx}"
                    )
                    xc_engs[tcx].dma_start(
                        xc[:], xT_r[:, :, tcx * 512 : (tcx + 1) * 512]
                    )
                for c in range(8):  # Q/K channel tiles
                    dstT = QT if c < 4 else KT
                    ps = psA.tile([128, 512], F32, tag="psA")
                    for kt in range(NKI):
                        nc.tensor.matmul(
                            ps[:],
                            wch(c, kt),
                            xc[:, kt],
                            start=(kt == 0),
                            stop=(kt == NKI - 1),
                        )
                    nc.vector.tensor_scalar_add(
                        dstT[:, c % 4, tcx * 512 : (tcx + 1) * 512],
                        ps[:],
                        bqk[c],
                    )
                for tt in range(4 * tcx, 4 * tcx + 4):  # V t-tiles
                    ps2 = psA.tile([128, DL], F32, tag="psA")
                    for kt in range(NKI):
                        nc.tensor.matmul(
                            ps2[:],
                            xc[:, kt, (tt % 4) * 128 : (tt % 4 + 1) * 128],
                            wv_chunk(kt),
                            start=(kt == 0),
                            stop=(kt == NKI - 1),
                        )
                    nc.vector.tensor_tensor(
                        V[:, tt, :, :DH],
                        ps2.rearrange("p (h d) -> p h d", h=HL),
                        bv.rearrange("p (h d) -> p h d", h=HL),
                        mybir.AluOpType.add,
                    )
                    nc.vector.tensor_copy(
                        V[:, tt, :, DH], ones1.to_broadcast((128, HL))
                    )

            out_ps[0] = psA
            tc_round(0)
            if causal:
                attn_chunk(0)  # qc0 only needs the diag psum pool
            tc_round(1)
            if causal:
                out_chunk(0)
                attn_chunk(1)
            tc_round(2)
            if causal:
                attn_chunk(2)
            tc_round(3)

        # ---- Phases B (qc=3) + C interleaved ----
        with (
            tc.tile_pool(name="spp", bufs=2, space="PSUM") as sp,
            tc.tile_pool(name="spc", bufs=1, space="PSUM") as spc,
            tc.tile_pool(name="ypB", bufs=1, space="PSUM") as ypb,
        ):
            sp_pool[0] = sp
            out_ps[0] = spc
            out_tag[0] = "spd"
            av_pools[0] = (yp, ypb)

            if not causal:
                attn_chunk(0)
                attn_chunk(1)
                attn_chunk(2)
                out_chunk(0)
                out_chunk(1)
                out_chunk(2)
            prev = scores_pair(0, 3)
            for p in range(HL // 2):
                nxt = scores_pair(p + 1, 3) if p + 1 < HL // 2 else None
                av_part(2 * p, 3, prev)
                av_part(2 * p + 1, 3, prev)
                if causal and p == 0:
                    out_chunk(1)
                if causal and p == 2:
                    out_chunk(2)
                prev = nxt
            out_chunk(3)


def _build(causal: bool, repeat: int = 1):
    nc = bacc.Bacc("TRN2", target_bir_lowering=False, debug=False)
    nc.dram_tensors = {}
    nc.dram_tensors["xT"] = nc.dram_tensor("xT", [D, T], BF16, kind="ExternalInput")
    nc.dram_tensors["w_qkv"] = nc.dram_tensor(
        "w_qkv", [128, 12288], BF16, kind="ExternalInput"
    )
    nc.dram_tensors["b_qk"] = nc.dram_tensor(
        "b_qk", [128, 8], F32, kind="ExternalInput"
    )
    nc.dram_tensors["b_v"] = nc.dram_tensor("b_v", [1, DL], F32, kind="ExternalInput")
    nc.dram_tensors["w_out"] = nc.dram_tensor(
        "w_out", [DL, D], BF16, kind="ExternalInput"
    )
    nc.dram_tensors["out"] = nc.dram_tensor("out", [T, D], BF16, kind="ExternalOutput")
    with tile.TileContext(nc) as tc:
        for _rep in range(repeat):
            _emit(nc, tc, causal)
    nc.compile()
    return nc


def _get_program(causal: bool):
    key = ("prog", causal)
    if key not in _CACHED:
        _CACHED[key] = _build(causal)
    return _CACHED[key]


def _run_fast(nc, causal, in_maps):
    """Execute via a cached jitted shard_map executable (avoids rebuilding
    the PJRT program on every call). Falls back to run_bass_kernel_spmd."""
    try:
        import jax
        from jax.sharding import Mesh, NamedSharding, PartitionSpec
        from jax.experimental.shard_map import shard_map
        from concourse import bass2jax
        from concourse.bass2jax import _bass_exec_p, install_neuronx_cc_hook

        key = ("exec", causal)
        if key not in _CACHED:
            install_neuronx_cc_hook()
            partition_name = (
                nc.partition_id_tensor.name if nc.partition_id_tensor else None
            )
            in_names, out_names, out_avals, zero_outs = [], [], [], []
            for alloc in nc.m.functions[0].allocations:
                if not isinstance(alloc, mybir.MemoryLocationSet):
                    continue
                name = alloc.memorylocations[0].name
                if alloc.kind == "ExternalInput":
                    if name != partition_name:
                        in_names.append(name)
                elif alloc.kind == "ExternalOutput":
                    out_names.append(name)
                    shape = tuple(alloc.tensor_shape)
                    dtype = mybir.dt.np(alloc.dtype)
                    out_avals.append(jax.core.ShapedArray(shape, dtype))
                    zero_outs.append(np.zeros(shape, dtype))
            n_params = len(in_names)
            in_names_full = in_names + out_names + (
                [partition_name] if partition_name else []
            )

            def _body(*args):
                operands = list(args)
                if partition_name is not None:
                    operands.append(bass2jax.partition_id_tensor())
                return tuple(
                    _bass_exec_p.bind(
                        *operands,
                        out_avals=tuple(out_avals),
                        in_names=tuple(in_names_full),
                        out_names=tuple(out_names),
                        lowering_input_output_aliases=(),
                        sim_require_finite=True,
                        sim_require_nnan=True,
                        nc=nc,
                    )
                )

            devices = jax.devices()[:8]
            mesh = Mesh(np.asarray(devices), ("core",))
            ex = jax.jit(
                shard_map(
                    _body,
                    mesh=mesh,
                    in_specs=(PartitionSpec("core"),) * (n_params + len(out_names)),
                    out_specs=(PartitionSpec("core"),) * len(out_names),
                    check_rep=False,
                ),
                keep_unused=True,
            )
            _CACHED[key] = (ex, in_names, zero_outs, mesh)
        ex, in_names, zero_outs, mesh = _CACHED[key]
        sh = NamedSharding(mesh, PartitionSpec("core"))
        concat_in = [
            np.concatenate([np.asarray(m[nm]) for m in in_maps], axis=0)
            for nm in in_names
        ]
        concat_zeros = [
            np.zeros((8 * z.shape[0], *z.shape[1:]), z.dtype) for z in zero_outs
        ]
        dev = [jax.device_put(a, sh) for a in concat_in + concat_zeros]
        out_arrs = ex(*dev)
        full = np.asarray(out_arrs[0]).reshape(8, T, D)
        return [full[c] for c in range(8)]
    except Exception:
        res = run_bass_kernel_spmd(nc, in_maps, list(range(8)))
        return [r["out"] for r in res.results]


def kernel(x, attn_mask, W_qkv, b_qkv, W_out, b_out, causal):
    from concourse import mybir as _mybir

    bf16 = _mybir.dt.np(_mybir.dt.bfloat16)
    x = np.asarray(x, dtype=np.float32)
    W_qkv = np.asarray(W_qkv, dtype=np.float32)
    b_qkv_np = np.asarray(b_qkv, dtype=np.float32)
    W_out = np.asarray(W_out, dtype=np.float32)
    b_out = np.asarray(b_out, dtype=np.float32)
    causal = bool(int(causal))

    nc = _get_program(causal)

    shards = []
    for g in range(2):
        w_shard = np.ascontiguousarray(
            np.concatenate(
                [
                    W_qkv[:, g * DL : (g + 1) * DL],
                    W_qkv[:, D + g * DL : D + (g + 1) * DL],
                    W_qkv[:, 2 * D + g * DL : 2 * D + (g + 1) * DL],
                ],
                axis=1,
            )
        )
        b_shard = np.ascontiguousarray(
            np.concatenate(
                [
                    b_qkv_np[g * DL : (g + 1) * DL],
                    b_qkv_np[D + g * DL : D + (g + 1) * DL],
                    b_qkv_np[2 * D + g * DL : 2 * D + (g + 1) * DL],
                ]
            )
        )
        # pre-arrange W_qkv into the on-chip layout: [ki, c, ko, 128] for the
        # 8 Q/K channel tiles, then [ki, ko, 512] for V (2KB-contiguous rows)
        ws3 = w_shard.reshape(8, 128, 3 * DL)
        qk = (
            ws3[:, :, : 2 * DL]
            .reshape(8, 128, 8, 128)
            .transpose(1, 2, 0, 3)
            .reshape(128, 2 * DL * 8)
        )
        vpart = ws3[:, :, 2 * DL :].transpose(1, 0, 2).reshape(128, DL * 8)
        warr = np.ascontiguousarray(np.concatenate([qk, vpart], axis=1)).astype(bf16)
        b_qk_arr = np.ascontiguousarray(b_shard[: 2 * DL].reshape(8, 128).T).astype(
            np.float32
        )
        b_v_arr = np.ascontiguousarray(b_shard[2 * DL :].reshape(1, DL)).astype(
            np.float32
        )
        wo_shard = np.ascontiguousarray(W_out[g * DL : (g + 1) * DL, :]).astype(bf16)
        shards.append((warr, b_qk_arr, b_v_arr, wo_shard))

    in_maps = []
    for c in range(8):
        b = c % B
        g = c // B
        warr, b_qk_arr, b_v_arr, wo_shard = shards[g]
        in_maps.append(
            {
                "xT": np.ascontiguousarray(x[b].T).astype(bf16),
                "w_qkv": warr,
                "b_qk": b_qk_arr,
                "b_v": b_v_arr,
                "w_out": wo_shard,
            }
        )

    outs = _run_fast(nc, causal, in_maps)
    y = np.empty((B, T, D), dtype=np.float32)
    for b in range(B):
        y[b] = (
            np.asarray(outs[b], dtype=np.float32)
            + np.asarray(outs[B + b], dtype=np.float32)
            + b_out
        )
    return y

